# revision 1
# baseline (speedup 1.0000x reference)
"""Trainium2 Bass kernel for nn_MultiDense (moe_routing) — v5.

Reference computation:
    p = params[inds_ne]            # [I, 128, 129] gathered per-index params
    w = p[..., :128]; b = p[..., 128]
    out[i] = x_in[i] @ w[i].T + b[i]     # [I, 32, 128]

v5 strategy (8 NeuronCores, data-parallel over I, params replicated):
  - bf16 end-to-end (tolerance 2e-2; measured error ~4e-3).
  - The v1/v2 bottleneck was per-index HWDGE dma_start issue overhead
    (~1.2us x 1024 = 1.3ms). v5 gathers 128 param blocks per ONE gpsimd
    dma_gather(transpose=True). Each table element is [129, 128]: the natural
    [l, k] weight block plus the bias vector as a 129th row. The gather's
    128-granular transpose lands it as [k(part), c, i] with c<128 the wT
    columns (matmul-rhs layout, no PE weight transposes) and c=128 the bias
    column.
  - Per quad: bias columns [128, 4] PE-transposed -> [4, 128], K=4 ind4
    matmul seeds PSUM with biases, then 4 col-tiled matmuls (lhsT = xT[:,32]
    stationary, rhs = wT[128,128] streamed at free-stride 128) accumulate
    into PSUM partition quarters. DVE copies PSUM -> bf16 SBUF.
"""
import numpy as np
import ml_dtypes
from contextlib import ExitStack

from concourse import bass, bacc, mybir
import concourse.tile as tile
from concourse.bass_utils import run_bass_kernel_spmd
from concourse.library_config import mlp

P = 128          # partitions / OUT_F / IN_F
V = 4096         # nodes
EC = 129         # rows per table element (128 wT cols + bias row)
E = EC * P       # elements per gathered block
J = 32           # samples per index
K = 128          # contraction size
I_FULL = 8192
N_CORES = 8
N_IDX = I_FULL // N_CORES   # per-core indices
CH = 128                     # indices per chunk (dma_gather num_idxs % 128)

BF16 = mybir.dt.bfloat16


def build_program(n_idx=N_IDX, ch=CH):
    nchunk = n_idx // ch
    nquad = ch // 4
    nc = bacc.Bacc("TRN2", target_bir_lowering=False, debug=False)
    wtab = nc.dram_tensor("wtab", [V, E], BF16, kind="ExternalInput")
    xt = nc.dram_tensor("xt", [nchunk, P, ch * J], BF16, kind="ExternalInput")
    widx = nc.dram_tensor(
        "widx", [P, n_idx // 16], mybir.dt.int16, kind="ExternalInput"
    )
    ident_in = nc.dram_tensor("ident", [P, P], BF16, kind="ExternalInput")
    ind4_in = nc.dram_tensor("ind4", [4, P], BF16, kind="ExternalInput")
    ydev = nc.dram_tensor("ydev", [nchunk, P, ch * J], BF16, kind="ExternalOutput")

    with tile.TileContext(nc) as tc:
        with ExitStack() as ctx:
            const = ctx.enter_context(tc.tile_pool(name="const", bufs=1))
            widx_t = const.tile([P, n_idx // 16], mybir.dt.int16)
            nc.sync.dma_start(widx_t[:], widx[:])
            ident = const.tile([P, P], BF16)
            nc.sync.dma_start(ident[:], ident_in[:])
            ind4 = const.tile([4, P], BF16)
            nc.sync.dma_start(ind4[:], ind4_in[:])

            nc.gpsimd.load_library(mlp)

            wtp = ctx.enter_context(tc.tile_pool(name="wtp", bufs=3))
            xtp = ctx.enter_context(tc.tile_pool(name="xtp", bufs=2))
            outp = ctx.enter_context(tc.tile_pool(name="outp", bufs=2))
            brp = ctx.enter_context(tc.tile_pool(name="brp", bufs=2))
            ps_y = ctx.enter_context(tc.tile_pool(name="ps_y", bufs=4, space="PSUM"))
            ps_b = ctx.enter_context(tc.tile_pool(name="ps_b", bufs=2, space="PSUM"))

            for c in range(nchunk):
                xt_tile = xtp.tile([P, ch * J], BF16, tag="xt")
                nc.sync.dma_start(xt_tile[:], xt[c])

                # gathered params: [k(part), c, i]; c<128 wT cols, c=128 bias.
                # The SWDGE descriptor ring holds ~256 descs/engine and a
                # transposed gather emits num_idxs*elem_bytes/256/16 rx-descs
                # per engine, so gather in 16-column slices (130 descs) using
                # elem_step to stride full table elements.
                wt = wtp.tile([P, EC * ch], BF16, tag="wt")
                idx_sl = widx_t[:, c * (ch // 16) : (c + 1) * (ch // 16)]
                SC = 16  # c-chunks per slice
                for s in range(EC // SC):
                    sl_ap = bass.AP(
                        wt[:].tensor,
                        wt[:].offset + s * SC * ch,
                        [wt[:].ap[0], [ch, SC], [1, ch]],
                    )
                    nc.gpsimd.dma_gather(
                        sl_ap,
                        wtab[:, s * SC * P : (s + 1) * SC * P],
                        idx_sl,
                        ch,
                        ch,
                        SC * P,
                        elem_step=E,
                        transpose=True,
                    )
                # bias row (c = 128): 1 desc/idx
                b_ap = bass.AP(
                    wt[:].tensor,
                    wt[:].offset + K * ch,
                    [wt[:].ap[0], [ch, 1], [1, ch]],
                )
                nc.gpsimd.dma_gather(
                    b_ap,
                    wtab[:, K * P :],
                    idx_sl,
                    ch,
                    ch,
                    P,
                    elem_step=E,
                    transpose=True,
                )

                yout = outp.tile([P, ch * J], BF16, tag="yo")
                for q in range(nquad):
                    bias_cols = bass.AP(
                        wt[:].tensor,
                        wt[:].offset + K * ch + 4 * q,
                        [wt[:].ap[0], [1, 4]],
                    )
                    biasT = ps_b.tile([4, P], BF16, tag="bt")
                    nc.tensor.transpose(biasT[:], bias_cols, ident[:])
                    biasq = brp.tile([4, P], BF16, tag="br")
                    nc.vector.tensor_copy(biasq[:], biasT[:])

                    ypsum = ps_y.tile([P, K], mybir.dt.float32, tag="yp")
                    nc.tensor.matmul(ypsum[:], ind4[:], biasq[:], start=True, stop=False)
                    for u in range(4):
                        t = q * 4 + u
                        rhs = bass.AP(
                            wt[:].tensor, wt[:].offset + t, [wt[:].ap[0], [ch, P]]
                        )
                        nc.tensor.matmul(
                            ypsum[32 * u : 32 * (u + 1), :],
                            xt_tile[:, t * J : (t + 1) * J],
                            rhs,
                            start=False,
                            stop=(u == 3),
                            tile_position=(0, 32 * u),
                        )
                    nc.vector.tensor_copy(yout[:, q * K : (q + 1) * K], ypsum[:])
                nc.sync.dma_start(ydev[c], yout[:])
    nc.compile()
    return nc


def make_consts():
    ident = np.eye(P, dtype=ml_dtypes.bfloat16)
    ind4 = np.zeros((4, P), ml_dtypes.bfloat16)
    for u in range(4):
        ind4[u, 32 * u : 32 * (u + 1)] = 1.0
    return ident, ind4


def make_tabs(params):
    """params [V, 128, 129] f32 -> wtab [V, 129*128] bf16: natural [l, k]
    weight block followed by the bias vector as a 129th row."""
    w = params[:, :, :K].reshape(V, P * K)
    b = params[:, :, K]
    return np.ascontiguousarray(np.concatenate([w, b], axis=1)).astype(
        ml_dtypes.bfloat16
    )


def wrap_idxs(ids):
    n = len(ids)
    w = np.asarray(ids, np.int16).reshape(n // 16, 16).T  # [16, n/16]
    return np.tile(w, (8, 1))  # [128, n/16]


def host_pre_core(x_core, inds_core, ch=CH):
    n = x_core.shape[0]
    nchunk = n // ch
    xt = np.ascontiguousarray(
        x_core.reshape(nchunk, ch, J, K).transpose(0, 3, 1, 2).reshape(nchunk, K, ch * J)
    ).astype(ml_dtypes.bfloat16)
    inds = inds_core.astype(np.int64)
    widx = np.hstack([wrap_idxs(inds[c * ch : (c + 1) * ch]) for c in range(nchunk)])
    return xt, widx.astype(np.int16)


def host_post_core(ydev, n, ch=CH):
    nchunk = n // ch
    nquad = ch // 4
    y = np.asarray(ydev).astype(np.float32).reshape(nchunk, 4, J, nquad, K)
    y = y.transpose(0, 3, 1, 2, 4)  # [c, q, u, j, l]
    return np.ascontiguousarray(y.reshape(n, J, K))


_NC_CACHE = {}


def get_program(n_idx=N_IDX, ch=CH):
    key = (n_idx, ch)
    if key not in _NC_CACHE:
        _NC_CACHE[key] = build_program(n_idx, ch)
    return _NC_CACHE[key]


def make_in_maps(x_in, inds_ne, params, n_cores=N_CORES, ch=CH):
    wtab = make_tabs(np.asarray(params, dtype=np.float32))
    ident, ind4 = make_consts()
    n_per = x_in.shape[0] // n_cores
    in_maps = []
    for cidx in range(n_cores):
        sl = slice(cidx * n_per, (cidx + 1) * n_per)
        xtc, widx = host_pre_core(np.asarray(x_in[sl]), np.asarray(inds_ne[sl]), ch)
        in_maps.append(
            {"wtab": wtab, "xt": xtc, "widx": widx, "ident": ident, "ind4": ind4}
        )
    return in_maps


def kernel(x_in, inds_ne, params):
    x_in = np.asarray(x_in, dtype=np.float32)
    inds_ne = np.asarray(inds_ne)
    params = np.asarray(params, dtype=np.float32)
    n_per = x_in.shape[0] // N_CORES

    nc = get_program(n_per, CH)
    in_maps = make_in_maps(x_in, inds_ne, params, N_CORES, CH)
    res = run_bass_kernel_spmd(nc, in_maps, core_ids=list(range(N_CORES)))
    outs = [host_post_core(res.results[c]["ydev"], n_per, CH) for c in range(N_CORES)]
    return np.concatenate(outs, axis=0)

